# revision 2
# baseline (speedup 1.0000x reference)
"""AffineNet v3: per-theta specialized programs, concurrently launched.

Each output channel o gets its own JIT-specialized Bass program (tight
per-theta chunk list + x-interval windows); the 7 live programs run
concurrently on 7 NeuronCores via async PJRT dispatch (different programs
per device — allowed because dispatch is asynchronous, no threads).

Math per 128-px chunk (px on PSUM partitions), hats fully on ScalarE via
per-partition bias tables:
  hy[y,px]=relu(1-|iy(px)-y|) (bf16), U[px,(b,x)] = hy^T @ X  (TensorE),
  out[px,b] = sum_x U*hx  (VectorE TT+segmented-reduce).
"""
import numpy as np

B, C, H, W = 8, 8, 256, 256
HW = H * W
P = 128
NCH = 512  # 128-px chunks; c = h*2 + wh

_cache = {}


def _theta6(theta):
    th6 = np.zeros((8, 8), np.float32)
    for o in range(8):
        t = theta[o].astype(np.float64)
        cx = 127.5 * (-t[0, 0] - t[0, 1] + t[0, 2] + 1)
        cy = 127.5 * (-t[1, 0] - t[1, 1] + t[1, 2] + 1)
        th6[o, :6] = [t[0, 0], t[0, 1], cx, t[1, 0], t[1, 1], cy]
    return th6


def _plan_o(th6, o):
    """Per-theta chunk plan: [(c, halves, subs)] with interval-set subs."""
    hs = np.arange(H, dtype=np.float64)
    ws = np.arange(W, dtype=np.float64)
    hh, ww = np.meshgrid(hs, ws, indexing="ij")
    t00, t01, cx, t10, t11, cy = th6[o, :6].astype(np.float64)
    ix = (t00 * ww + t01 * hh + cx).reshape(NCH, P)
    iy = (t10 * ww + t11 * hh + cy).reshape(NCH, P)
    inb = (ix > -1) & (ix < 256) & (iy > -1) & (iy < 256)
    plan = []
    for c in range(NCH):
        m = inb[c]
        if not m.any():
            continue
        a = ix[c][m]; b = iy[c][m]
        xl = int(np.clip(np.floor(a.min()), 0, 255))
        xh = int(np.clip(np.floor(a.max()) + 1, 0, 255))
        yl = int(np.clip(np.floor(b.min()), 0, 255))
        yh = int(np.clip(np.floor(b.max()) + 1, 0, 255))
        halves = [r for r in (0, 1) if yl <= 128 * r + 127 and yh >= 128 * r]
        span = xh - xl + 1

        def cls(s):
            return 32 if s <= 32 else (64 if s <= 64 else 128)
        if span <= 128:
            sizes = [cls(span)]
        else:
            sizes = [128, cls(span - 128)]
        tot = sum(sizes)
        lo2 = min(xl, 256 - tot)
        subs = []
        off = lo2
        for S in sizes:
            subs.append((off, S))
            off += S
        plan.append((c, halves, subs))
    return plan


def _build(plan, repeat):
    import concourse.bacc as bacc
    import concourse.bass as bass
    import concourse.mybir as mybir
    import concourse.tile as tile
    f32 = mybir.dt.float32
    bf16 = mybir.dt.bfloat16
    Alu = mybir.AluOpType
    Act = mybir.ActivationFunctionType

    nc = bacc.Bacc("TRN2", target_bir_lowering=False, debug=False)
    xbd = nc.dram_tensor("xb", [B * C, HW], bf16, kind="ExternalInput")
    thtd = nc.dram_tensor("tht", [P, 8], f32, kind="ExternalInput")
    piod = nc.dram_tensor("pio", [P, 4], f32, kind="ExternalInput")  # p,-p,-(p+128),1
    u128d = nc.dram_tensor("u128", [P, P], f32, kind="ExternalInput")
    u256d = nc.dram_tensor("u256", [P, 256], f32, kind="ExternalInput")
    whd = nc.dram_tensor("whm", [P, 2 * NCH], f32, kind="ExternalInput")  # wmap|hmap
    eyed = nc.dram_tensor("eye", [P, P], f32, kind="ExternalInput")
    res = nc.dram_tensor("res", [B, HW], f32, kind="ExternalOutput")

    def apx(t, off, dims):
        return bass.AP(t.tensor, t.offset + off, [t.ap[0]] + dims)

    with tile.TileContext(nc) as tc:
        with (
            tc.tile_pool(name="cst", bufs=1) as cpool,
            tc.tile_pool(name="xst", bufs=1) as xpool,
            tc.tile_pool(name="wrk", bufs=1) as wpool,
            tc.tile_pool(name="ups", bufs=2, space="PSUM") as upool,
            tc.tile_pool(name="tps", bufs=2, space="PSUM") as tpool,
            tc.tile_pool(name="out", bufs=2) as opool,
        ):
            tht = cpool.tile([P, 8], f32, name="tht")
            pio = cpool.tile([P, 4], f32, name="pio")
            u128 = cpool.tile([P, P], f32, name="u128")
            u256 = cpool.tile([P, 256], f32, name="u256")
            whm = cpool.tile([P, 2 * NCH], f32, name="whm")
            eye = cpool.tile([P, P], f32, name="eye")
            nc.sync.dma_start(tht[:], thtd.ap())
            nc.sync.dma_start(pio[:], piod.ap())
            nc.sync.dma_start(u128[:], u128d.ap())
            nc.sync.dma_start(u256[:], u256d.ap())
            nc.sync.dma_start(whm[:], whd.ap())
            nc.sync.dma_start(eye[:], eyed.ap())

            cxa = cpool.tile([P, NCH], f32, name="cxa")
            cya = cpool.tile([P, NCH], f32, name="cya")
            nixa = cpool.tile([P, NCH], f32, name="nixa")
            cyp0 = cpool.tile([P, NCH], f32, name="cyp0")
            cyp1 = cpool.tile([P, NCH], f32, name="cyp1")
            t10u = cpool.tile([P, P], f32, name="t10u")
            pp0 = cpool.tile([P, 1], f32, name="pp0")
            Xh = [cpool.tile([P, 8 * 256], bf16, name=f"Xh{r}") for r in range(2)]
            Rt = cpool.tile([P, 8 * NCH], f32, name="Rt")

            with tc.For_i(0, repeat):
                # ---- theta-derived tables ----
                tmp = wpool.tile([P, NCH], f32, tag="tmp", name="tmp")
                nc.vector.tensor_scalar(cxa[:], whm[:, :NCH], tht[:, 0:1], None, op0=Alu.mult)
                nc.vector.tensor_scalar(tmp[:], whm[:, NCH:], tht[:, 1:2], tht[:, 2:3],
                                        op0=Alu.mult, op1=Alu.add)
                nc.vector.tensor_add(cxa[:], cxa[:], tmp[:])
                nc.vector.tensor_scalar(cya[:], whm[:, :NCH], tht[:, 3:4], None, op0=Alu.mult)
                nc.vector.tensor_scalar(tmp[:], whm[:, NCH:], tht[:, 4:5], tht[:, 5:6],
                                        op0=Alu.mult, op1=Alu.add)
                nc.vector.tensor_add(cya[:], cya[:], tmp[:])
                # nixa = -(t00*p + cxa) ; cyp_r = cya - p - 128r ; t10u = t10*f
                nc.vector.tensor_scalar(pp0[:], pio[:, 0:1], tht[:, 0:1], None, op0=Alu.mult)
                nc.vector.tensor_scalar(nixa[:], cxa[:], -1.0, None, op0=Alu.mult)
                nc.vector.tensor_scalar(nixa[:], nixa[:], pp0[:], None, op0=Alu.subtract)
                nc.vector.tensor_scalar(cyp0[:], cya[:], pio[:, 1:2], None, op0=Alu.add)
                nc.vector.tensor_scalar_add(cyp1[:], cyp0[:], -128.0)
                nc.vector.tensor_scalar(t10u[:], u128[:], tht[:, 3:4], None, op0=Alu.mult)

                # ---- xbar = mean_c x ----
                for r in range(2):
                    xc = [xpool.tile([P, 8 * 256], bf16, tag=f"xc{i}", name=f"xc{i}")
                          for i in range(8)]
                    for ch in range(8):
                        src = bass.AP(xbd.ap().tensor, ch * HW + r * P * W,
                                      [[W, P], [C * HW, 8], [1, W]])
                        nc.sync.dma_start(xc[ch][:], src)
                    s01 = xpool.tile([P, 8 * 256], f32, tag="s01", name="s01")
                    s23 = xpool.tile([P, 8 * 256], f32, tag="s23", name="s23")
                    s45 = xpool.tile([P, 8 * 256], f32, tag="s45", name="s45")
                    s67 = xpool.tile([P, 8 * 256], f32, tag="s67", name="s67")
                    nc.vector.tensor_add(s01[:], xc[0][:], xc[1][:])
                    nc.vector.tensor_add(s23[:], xc[2][:], xc[3][:])
                    nc.vector.tensor_add(s45[:], xc[4][:], xc[5][:])
                    nc.vector.tensor_add(s67[:], xc[6][:], xc[7][:])
                    nc.vector.tensor_add(s01[:], s01[:], s23[:])
                    nc.vector.tensor_add(s45[:], s45[:], s67[:])
                    nc.vector.tensor_add(s01[:], s01[:], s45[:])
                    nc.vector.tensor_scalar_mul(Xh[r][:], s01[:], 1.0 / C)

                nc.vector.memset(Rt[:], 0.0)

                # ---- chunk loop ----
                cyps = [cyp0, cyp1]
                for (c, halves, subs) in plan:
                    hys = {}
                    for r in halves:
                        hy = wpool.tile([P, P], bf16, tag=f"hy{r}", name=f"hy{r}", bufs=3)
                        nc.scalar.activation(hy[:], t10u[:], Act.Abs,
                                             bias=cyps[r][:, c:c + 1])
                        nc.scalar.activation(hy[:], hy[:], Act.Relu, bias=1.0, scale=-1.0)
                        hys[r] = hy
                    first_sub = True
                    for (lo, S) in subs:
                        U = upool.tile([P, 1024], f32, tag="U", name="U")
                        nmm = 1 if 8 * S <= 512 else 2
                        nb = 8 // nmm
                        for i, r in enumerate(halves):
                            st = (i == 0)
                            sp = (i == len(halves) - 1)
                            for j in range(nmm):
                                rhs = apx(Xh[r][:], j * nb * 256 + lo, [[256, nb], [1, S]])
                                nc.tensor.matmul(U[:, j * nb * S:(j + 1) * nb * S],
                                                 hys[r][:], rhs, start=st, stop=sp)
                        hx = wpool.tile([P, P], f32, tag="hx", name="hx", bufs=3)
                        nc.scalar.activation(hx[:, :S], u256[:, lo:lo + S], Act.Abs,
                                             bias=nixa[:, c:c + 1])
                        nc.scalar.activation(hx[:, :S], hx[:, :S], Act.Relu,
                                             bias=1.0, scale=-1.0)
                        M = wpool.tile([P, 1024], f32, tag="M", name="M", bufs=2)
                        nc.vector.tensor_tensor(
                            apx(M[:], 0, [[S, 8], [1, S]]),
                            apx(U[:], 0, [[S, 8], [1, S]]),
                            apx(hx[:], 0, [[0, 8], [1, S]]), op=Alu.mult)
                        if first_sub:
                            nc.vector.tensor_reduce(
                                apx(Rt[:], c, [[NCH, 8]]),
                                apx(M[:], 0, [[S, 8], [1, S]]),
                                axis=mybir.AxisListType.X, op=Alu.add)
                        else:
                            t8 = wpool.tile([P, 8], f32, tag="t8", name="t8", bufs=2)
                            nc.vector.tensor_reduce(
                                t8[:], apx(M[:], 0, [[S, 8], [1, S]]),
                                axis=mybir.AxisListType.X, op=Alu.add)
                            nc.vector.tensor_tensor(
                                apx(Rt[:], c, [[NCH, 8]]),
                                apx(Rt[:], c, [[NCH, 8]]), t8[:], op=Alu.add)
                        first_sub = False

                # ---- transpose + store ----
                for b in range(8):
                    tp = tpool.tile([P, 512], f32, tag="tp", name="tp")
                    for k in range(4):
                        nc.tensor.transpose(tp[:, k * P:(k + 1) * P],
                                            Rt[:, b * NCH + k * P:b * NCH + (k + 1) * P],
                                            eye[:])
                    tsb = opool.tile([P, 512], f32, tag="tsb", name="tsb")
                    nc.vector.tensor_copy(tsb[:], tp[:])
                    dst = bass.AP(res.ap().tensor, b * HW,
                                  [[P, P], [P * P, 4], [1, P]])
                    nc.sync.dma_start(dst, tsb[:])
    nc.compile()
    return nc


def _consts():
    pio = np.zeros((P, 4), np.float32)
    pio[:, 0] = np.arange(P)
    pio[:, 1] = -np.arange(P)
    pio[:, 2] = -(np.arange(P) + 128.0)
    pio[:, 3] = 1.0
    u128 = np.broadcast_to(np.arange(P, dtype=np.float32), (P, P)).copy()
    u256 = np.broadcast_to(np.arange(256, dtype=np.float32), (P, 256)).copy()
    wmap = np.array([128.0 * (c % 2) for c in range(NCH)], np.float32)
    hmap = np.array([float(c // 2) for c in range(NCH)], np.float32)
    whm = np.broadcast_to(np.concatenate([wmap, hmap]), (P, 2 * NCH)).copy()
    eye = np.eye(P, dtype=np.float32)
    return pio, u128, u256, whm, eye


def _make_launcher(nc):
    import jax
    import concourse.mybir as mybir
    from concourse.bass2jax import (_bass_exec_p, install_neuronx_cc_hook,
                                    partition_id_tensor)
    install_neuronx_cc_hook()
    partition_name = nc.partition_id_tensor.name if nc.partition_id_tensor else None
    in_names, out_names, out_avals, zero_outs = [], [], [], []
    for alloc in nc.m.functions[0].allocations:
        if not isinstance(alloc, mybir.MemoryLocationSet):
            continue
        name = alloc.memorylocations[0].name
        if alloc.kind == "ExternalInput":
            if name != partition_name:
                in_names.append(name)
        elif alloc.kind == "ExternalOutput":
            out_names.append(name)
            shape = tuple(alloc.tensor_shape)
            dtype = mybir.dt.np(alloc.dtype)
            out_avals.append(jax.core.ShapedArray(shape, dtype))
            zero_outs.append(np.zeros(shape, dtype))
    n_params = len(in_names)
    all_names = list(in_names) + out_names
    if partition_name is not None:
        all_names.append(partition_name)
    donate = tuple(range(n_params, n_params + len(out_names)))

    def _body(*args):
        operands = list(args)
        if partition_name is not None:
            operands.append(partition_id_tensor())
        outs = _bass_exec_p.bind(
            *operands, out_avals=tuple(out_avals), in_names=tuple(all_names),
            out_names=tuple(out_names),
            lowering_input_output_aliases=(), sim_require_finite=True,
            sim_require_nnan=True, nc=nc)
        return tuple(outs)

    jitted = jax.jit(_body, donate_argnums=donate, keep_unused=True)

    def launch(in_map, device):
        args = [jax.device_put(np.asarray(in_map[n]), device) for n in in_names]
        args += [jax.device_put(z.copy(), device) for z in zero_outs]
        return jitted(*args), out_names

    return launch


def _get_programs(theta, repeat):
    th6 = _theta6(theta)
    progs = {}
    for o in range(8):
        plan = _plan_o(th6, o)
        if not plan:
            continue
        key = (th6[o].tobytes(), repeat)
        if key not in _cache:
            nc = _build(plan, repeat)
            _cache[key] = (nc, _make_launcher(nc))
        progs[o] = _cache[key]
    return th6, progs


def _in_map(th6, o, xb, consts):
    pio, u128, u256, whm, eye = consts
    tt = np.broadcast_to(th6[o], (P, 8)).copy()
    return {"xb": xb, "tht": tt, "pio": pio, "u128": u128,
            "u256": u256, "whm": whm, "eye": eye}


def kernel(x, theta):
    import jax
    import ml_dtypes
    x = np.ascontiguousarray(x, dtype=np.float32)
    theta = np.ascontiguousarray(theta, dtype=np.float32)
    th6, progs = _get_programs(theta, 1)
    xb = x.reshape(B * C, HW).astype(ml_dtypes.bfloat16)
    consts = _consts()
    devs = jax.devices()
    futs = {}
    for o, (nc, launch) in progs.items():
        futs[o] = launch(_in_map(th6, o, xb, consts), devs[o])
    out = np.zeros((B, B, H, W), np.float32)
    for o, (arrs, onames) in futs.items():
        res = np.asarray(arrs[onames.index("res")])
        out[:, o] = res.reshape(B, H, W)
    return out
